# revision 1
# baseline (speedup 1.0000x reference)
"""Trainium2 Bass kernel for nn_Discriminator_55800215109843.

Model: 4x (Conv2d k3 s2 p1 + LeakyReLU(0.2) [+ BatchNorm eval]) on
[128,3,128,128] -> [128,128,8,8], then a 50-step LIF neuron scan
(beta=0.95, thr=1, subtract reset) whose spike record feeds a linear
layer [409600 -> 1] + sigmoid.

Strategy (8 NeuronCores, pure data parallelism over batch, 16 imgs/core):
  * Convs as tap-accumulation matmuls: channels (x images, block-diag
    weights) on the contraction dim, strided access-pattern views of
    zero-padded SBUF planes for the taps; PSUM accumulation.
  * All conv matmuls run as fp16 hi/lo splits: x = xh + xl, w = wh + wl
    (fp16 each), conv = wh*xh + wh*xl + wl*xh accumulated in fp32 PSUM.
    3 fp16 matmuls (1 cyc/row) replace 1 fp32 matmul (4 cyc/row); the
    dropped wl*xl term is ~2^-22 relative -- measured end-to-end error
    0.0034 vs fp32's 0.0032 (the LIF output is extremely sensitive to
    conv noise, so bf16/f32r/plain-fp16 convs are all out of budget).
  * L1 uses dy-replicated input planes (even rows only, 3 partition
    blocks of (img8, ch3)) so dy rides the contraction dim: 3 dx taps
    instead of 9, K=72, full PE array.
  * LeakyReLU(0.2) via lrelu(x) = x - 0.8*relu(-x): ACT Relu pass +
    one fused DVE scalar_tensor_tensor pass. BN (eval) is folded into
    conv weights/biases on the host.
  * LIF scan in layout [c=128 partitions, (b=16,hw=64) free]: 2 fused
    DVE STT passes per step (u = 0.95*m + (c-0.5); m = u - 0.5*r), the
    spike sign r = sign(m-1) on the otherwise-idle ACT engine, and the
    linear layer folded INTO the scan as 50 accumulating PE matmuls
    (float32r, full rate) against the +-1 r tiles; the hw-diagonal of
    the [64,1024] PSUM result plus the sum-of-wl constant recover the
    0/1-spike dot product on the host.
  * Device matmuls avoid rapidly alternating tile_position row bases
    (0 <-> 64) -- that pattern hard-crashes the device; each layer
    issues all base-0 groups, then all base-64 groups.
"""

import sys

sys.path.insert(0, "/opt/trn_rl_repo")

import numpy as np

import concourse.bass as bass
import concourse.mybir as mybir
import concourse.tile as tile
from concourse import bacc
from concourse.bass_utils import run_bass_kernel_spmd

F32 = mybir.dt.float32
F16 = mybir.dt.float16
F32R = mybir.dt.float32r
OP = mybir.AluOpType
AF = mybir.ActivationFunctionType

N_CORES = 8
B_FULL = 128
B_LOC = 16          # images per core
T = 50              # LIF steps
BETA = 0.95
S = 128             # input spatial


def _np(x):
    return np.ascontiguousarray(np.asarray(x, dtype=np.float32))


def _fold_bn(g, bb, rm, rv, eps=0.8):
    scale = g / np.sqrt(rv + eps)
    shift = bb - rm * scale
    return scale.astype(np.float32), shift.astype(np.float32)


def _split16(a):
    """fp32 array -> (hi, lo) fp16 pair with hi + lo ~= a (to ~2^-21)."""
    hi = a.astype(np.float16)
    lo = (a - hi.astype(np.float32)).astype(np.float16)
    return np.ascontiguousarray(hi), np.ascontiguousarray(lo)


def _block_diag_taps(w, n_img, col_scale=None):
    """w: [C_out, C_in, 3, 3] -> taps [9, 128, 128] block-diag over n_img
    images, duplicated at row offset 64 for tile_position row pairing.

    rows: 64*h + (i_loc*C_in + c)   (h in {0,1} duplicate halves)
    cols: i_loc*C_out + c_out
    """
    co, ci = w.shape[0], w.shape[1]
    k = n_img * ci
    m = n_img * co
    assert k <= 64 or n_img == 1, (k, n_img)
    assert m <= 128
    taps = np.zeros((9, 128, 128), np.float32)
    for tp in range(9):
        dy, dx = tp // 3, tp % 3
        blk = w[:, :, dy, dx].T.astype(np.float32)  # [ci, co]
        if col_scale is not None:
            blk = blk * col_scale[None, :]
        for i in range(n_img):
            taps[tp, i * ci : (i + 1) * ci, i * co : (i + 1) * co] = blk
        if k <= 64:
            taps[tp, 64 : 64 + k, :] = taps[tp, :k, :]
    return taps


def _l1_dyrep_taps(w):
    """w1 [16, 3, 3, 3] -> dx-taps [3, 128, 128], rows (dy*24 + i*3 + c),
    cols (i*16 + c_out), block-diag over 8 images."""
    taps = np.zeros((3, 128, 128), np.float32)
    for dx in range(3):
        for dy in range(3):
            blk = w[:, :, dy, dx].T.astype(np.float32)  # [3, 16]
            for i in range(8):
                taps[dx, dy * 24 + i * 3 : dy * 24 + i * 3 + 3,
                     i * 16 : (i + 1) * 16] = blk
    return taps


def _bias_vec(b, n_img):
    v = np.zeros((128, 1), np.float32)
    co = b.shape[0]
    for i in range(n_img):
        v[i * co : (i + 1) * co, 0] = b
    return v


def build_nc():
    nc = bacc.Bacc("TRN2", target_bir_lowering=False, debug=False)

    # ---------------- DRAM I/O ----------------
    imgh_d = nc.dram_tensor("imgh", [B_LOC, 3, S, S], F16, kind="ExternalInput")
    imgl_d = nc.dram_tensor("imgl", [B_LOC, 3, S, S], F16, kind="ExternalInput")
    w_d = {}
    w_d[1] = nc.dram_tensor("w1t", [2, 3, 128, 128], F16, kind="ExternalInput")
    for li in (2, 3, 4):
        w_d[li] = nc.dram_tensor(f"w{li}t", [2, 9, 128, 128], F16, kind="ExternalInput")
    bp_d = nc.dram_tensor("biasp", [4, 128], F32, kind="ExternalInput")  # for DVE pass
    bn_d = nc.dram_tensor("biasn", [4, 128], F32, kind="ExternalInput")  # -0.8*b for ACT
    wl_d = nc.dram_tensor("wlt", [128, T * 64], F32R, kind="ExternalInput")
    out_d = nc.dram_tensor("D", [64, 1024], F32, kind="ExternalOutput")

    with tile.TileContext(nc) as tc:
        with (
            tc.tile_pool(name="const", bufs=1) as constp,
            tc.tile_pool(name="acts", bufs=1) as acts,
            tc.tile_pool(name="tmps", bufs=4) as tmps,
            tc.tile_pool(name="psum", bufs=4, space="PSUM") as psp,
            tc.tile_pool(name="psl4", bufs=1, space="PSUM") as psl4,
        ):
            # ---------------- load constants ----------------
            # (only L1 weights + biases here; the bulky L2-L4/wl loads are
            # issued AFTER the img DMAs so they don't delay the L1 start)
            wsb = {}
            wsb[1] = constp.tile([128, 2, 3, 128], F16, name="w1sb", tag="w1sb")
            nc.sync.dma_start(wsb[1][:], w_d[1].ap().transpose([2, 0, 1, 3]))
            biasp = constp.tile([128, 4], F32, name="biasp", tag="biasp")
            nc.sync.dma_start(biasp[:], bp_d.ap().transpose([1, 0]))
            biasn = constp.tile([128, 4], F32, name="biasn", tag="biasn")
            nc.sync.dma_start(biasn[:], bn_d.ap().transpose([1, 0]))

            # ---------------- activation planes (all fp16 hi/lo pairs) ----
            # x1: dy-replicated, even rows only. Per group g tiles
            # [72=(dy3,img8,ch3), 64*130]: block dy, row y holds padded row
            # 2y+dy (img row 2y+dy-1): L1 needs only 3 dx taps, K=72.
            # x2: 2 tiles [128=(i8,c16), 66*66]
            # x3: 4 tiles [128=(i4,c32), 34*34]
            # x4: 8 tiles [128=(i2,c64), 18*18]
            def hl(shape, nm, n):
                h = [acts.tile(shape, F16, name=f"{nm}h{i}", tag=f"{nm}h{i}") for i in range(n)]
                l = [acts.tile(shape, F16, name=f"{nm}l{i}", tag=f"{nm}l{i}") for i in range(n)]
                return h, l

            x1h, x1l = hl([72, 64 * 130], "x1", 2)
            x2h, x2l = hl([128, 66 * 66], "x2", 2)
            x3h, x3l = hl([128, 34 * 34], "x3", 4)
            x4h, x4l = hl([128, 18 * 18], "x4", 8)
            ctile = acts.tile([128, 1024], F32, name="ctile", tag="ctile")

            def zero_borders(t, npart, hp):
                v = t[:].rearrange("p (h w) -> p h w", w=hp)[0:npart]
                nc.gpsimd.memset(v[:, 0, :], 0.0)
                nc.gpsimd.memset(v[:, hp - 1, :], 0.0)
                nc.gpsimd.memset(v[:, 1 : hp - 1, 0], 0.0)
                nc.gpsimd.memset(v[:, 1 : hp - 1, hp - 1], 0.0)

            for g in range(2):
                for t in (x1h[g], x1l[g]):
                    v = t[:].rearrange("p (h w) -> p h w", w=130)
                    nc.gpsimd.memset(v[0:24, 0, :], 0.0)     # dy=0, y=0: pad row
                    nc.gpsimd.memset(v[0:72, :, 0], 0.0)     # left pad col
                    nc.gpsimd.memset(v[0:72, :, 129], 0.0)   # right pad col
            for t in x2h + x2l:
                zero_borders(t, 128, 66)
            for t in x3h + x3l:
                zero_borders(t, 128, 34)
            for t in x4h + x4l:
                zero_borders(t, 128, 18)

            # ---------------- input DMA ----------------
            # block dy, row y <- img row (2y + dy - 1); dy=0 skips y=0 (pad)
            for g in range(2):
                for src_d, xt in ((imgh_d, x1h), (imgl_d, x1l)):
                    v = xt[g][:].rearrange("p (h w) -> p h w", w=130)
                    for dy in range(3):
                        y0 = 1 if dy == 0 else 0
                        ny = 64 - y0
                        srcap = bass.AP(
                            tensor=src_d,
                            offset=g * 8 * 3 * S * S + (2 * y0 + dy - 1) * S,
                            ap=[[S * S, 24], [2 * S, ny], [1, S]],
                        )
                        nc.sync.dma_start(v[24 * dy : 24 * dy + 24, y0:64, 1:129], srcap)

            # deferred bulky constant loads (behind the img planes)
            for li in (2, 3, 4):
                wsb[li] = constp.tile([128, 2, 9, 128], F16, name=f"w{li}sb", tag=f"w{li}sb")
                nc.sync.dma_start(wsb[li][:], w_d[li].ap().transpose([2, 0, 1, 3]))
            wl = constp.tile([128, T * 64], F32R, name="wl", tag="wl")
            nc.sync.dma_start(wl[:], wl_d.ap())

            # ---------------- conv layers ----------------
            def conv_layer(wtile, rhs_of, psum_sets, emit_out, ntaps=9):
                """Split-fp16 tap-accumulation conv.

                wtile: [128, 2, ntaps, 128] (hi/lo on axis 1)
                rhs_of(gi, tap, q, sel) -> (rhs AP from hi/lo plane, tpos|None)
                psum_sets: list of (gi, q) output chunk ids
                emit_out(gi, q, ps_flat): epilogue on filled psum slice
                """
                terms = [(tp, wsel, xsel) for tp in range(ntaps)
                         for wsel, xsel in ((0, 0), (0, 1), (1, 0))]
                for gi, q in psum_sets:
                    ps = psp.tile([128, 512], F32, name="convps", tag="convps")
                    n = None
                    for idx, (tp, wsel, xsel) in enumerate(terms):
                        rhs, tpos = rhs_of(gi, tp, q, xsel)
                        kk = rhs.partition_size()
                        n = rhs.free_size()
                        base = tpos[0] if tpos is not None else 0
                        lhsT = wtile[base : base + kk, wsel, tp, :]
                        nc.tensor.matmul(
                            ps[:, 0:n],
                            lhsT,
                            rhs,
                            start=(idx == 0),
                            stop=(idx == len(terms) - 1),
                            tile_position=tpos,
                        )
                    emit_out(gi, q, ps[:, 0:n])

            def epilogue_split(ps, out_hi, out_lo, bias_idx):
                """x = lrelu(ps + bias) = (ps+b) + 0.8*relu(-(ps+b));
                out_hi = fp16(x), out_lo = fp16(x - out_hi)."""
                n_free = ps.free_size()
                r = tmps.tile([128, 512], F32, name="relu_tmp", tag="relu_tmp")
                rr = r[:, 0:n_free]
                xf = tmps.tile([128, 512], F32, name="xf_tmp", tag="xf_tmp")
                xF = xf[:, 0:n_free]
                nc.scalar.activation(
                    rr, ps, AF.Relu,
                    bias=biasn[:, bias_idx : bias_idx + 1], scale=-0.8,
                )
                nc.vector.scalar_tensor_tensor(
                    xF, ps, biasp[:, bias_idx : bias_idx + 1], rr, OP.add, OP.add
                )
                nc.scalar.activation(out_hi, xF, AF.Copy)
                nc.vector.scalar_tensor_tensor(
                    out_lo, out_hi, -1.0, xF, OP.mult, OP.add
                )

            def epilogue_final(ps, out_ap, bias_idx):
                """fp32 lrelu epilogue (L4 -> ctile)."""
                n_free = ps.free_size()
                r = tmps.tile([128, 512], F32, name="relu_tmp", tag="relu_tmp")
                rr = r[:, 0:n_free]
                nc.scalar.activation(
                    rr, ps, AF.Relu,
                    bias=biasn[:, bias_idx : bias_idx + 1], scale=-0.8,
                )
                nc.vector.scalar_tensor_tensor(
                    out_ap, ps, biasp[:, bias_idx : bias_idx + 1], rr, OP.add, OP.add
                )

            # ---- L1: groups g in {0,1} (8 imgs), 8 col chunks of 512,
            # dy-replicated input -> 3 dx taps, K=72, full PE array ----
            def l1_rhs(g, dx, q, sel):
                xt = (x1h, x1l)[sel]
                v = xt[g][:].rearrange("p (h w) -> p h w", w=130)
                return v[0:72, 8 * q : 8 * q + 8, dx : dx + 128 : 2], None

            def l1_out(g, q, ps):
                # psum [128=(i8,co16), (yy8, x64)] -> x2 interior rows 8q..8q+8
                def dst(xt):
                    return xt[g][:].rearrange("p (h w) -> p h w", w=66)[
                        :, 8 * q + 1 : 8 * q + 9, 1:65
                    ]
                epilogue_split(ps, dst(x2h), dst(x2l), 0)

            conv_layer(
                wsb[1], l1_rhs, [(g, q) for g in range(2) for q in range(8)], l1_out,
                ntaps=3,
            )

            # ---- L2: groups g2 in {0..3} (4 imgs), 2 col chunks of 512 ----
            def l2_rhs(g2, tp, q, sel):
                dy, dx = tp // 3, tp % 3
                xt = (x2h, x2l)[sel]
                v = xt[g2 // 2][:].rearrange("p (h w) -> p h w", w=66)
                base = 64 * (g2 % 2)
                rows = 32 * q + dy
                rhs = v[base : base + 64, rows : rows + 32 : 2, dx : dx + 64 : 2]
                return rhs, (base, 0)

            def l2_out(g2, q, ps):
                def dst(xt):
                    return xt[g2][:].rearrange("p (h w) -> p h w", w=34)[
                        :, 16 * q + 1 : 16 * q + 17, 1:33
                    ]
                epilogue_split(ps, dst(x3h), dst(x3l), 1)

            conv_layer(
                wsb[2], l2_rhs,
                [(g, q) for g in (0, 2, 1, 3) for q in range(2)], l2_out
            )

            # ---- L3: groups g3 in {0..7} (2 imgs), one 256-col chunk ----
            def l3_rhs(g3, tp, q, sel):
                dy, dx = tp // 3, tp % 3
                xt = (x3h, x3l)[sel]
                v = xt[g3 // 2][:].rearrange("p (h w) -> p h w", w=34)
                base = 64 * (g3 % 2)
                rhs = v[base : base + 64, dy : dy + 32 : 2, dx : dx + 32 : 2]
                return rhs, (base, 0)

            def l3_out(g3, q, ps):
                def dst(xt):
                    return xt[g3][:].rearrange("p (h w) -> p h w", w=18)[
                        :, 1:17, 1:17
                    ]
                epilogue_split(ps, dst(x4h), dst(x4l), 2)

            conv_layer(
                wsb[3], l3_rhs, [(g, 0) for g in (0, 2, 4, 6, 1, 3, 5, 7)], l3_out
            )

            # ---- L4: 16 imgs, 64 cols each, 2 long-lived psum banks ----
            ps4 = [psl4.tile([128, 512], F32, name=f"ps4_{i}", tag=f"ps4_{i}") for i in range(2)]
            for ii in [0, 2, 4, 6, 8, 10, 12, 14, 1, 3, 5, 7, 9, 11, 13, 15]:
                base = 64 * (ii % 2)
                terms = [(tp, wsel, xsel) for tp in range(9)
                         for wsel, xsel in ((0, 0), (0, 1), (1, 0))]
                for idx, (tp, wsel, xsel) in enumerate(terms):
                    dy, dx = tp // 3, tp % 3
                    xt = (x4h, x4l)[xsel]
                    v = xt[ii // 2][:].rearrange("p (h w) -> p h w", w=18)
                    rhs = v[base : base + 64, dy : dy + 16 : 2, dx : dx + 16 : 2]
                    lhsT = wsb[4][base : base + 64, wsel, tp, :]
                    nc.tensor.matmul(
                        ps4[ii // 8][:, 64 * (ii % 8) : 64 * (ii % 8) + 64],
                        lhsT,
                        rhs,
                        start=(idx == 0),
                        stop=(idx == len(terms) - 1),
                        tile_position=(base, 0),
                        skip_group_check=True,
                    )
            for pb in range(2):
                epilogue_final(ps4[pb][:], ctile[:, 512 * pb : 512 * pb + 512], 3)

            # ---------------- LIF scan + folded linear ----------------
            # Two fused DVE STT passes per step; ACT sign and the PE
            # d-matmuls hide under the next step's DVE work. (A one-pass
            # variant keeping cp-0.5r in PSUM via diag f32r matmuls was
            # tried and lost: the scan's PE matmuls run at mid-pstate and
            # cost more than the saved DVE pass.)
            with (
                tc.tile_pool(name="scan", bufs=1) as scp,
                tc.tile_pool(name="psd", bufs=1, space="PSUM") as psd,
            ):
                m = scp.tile([128, 1024], F32, name="m", tag="m")
                u = scp.tile([128, 1024], F32, name="u", tag="u")
                cp = scp.tile([128, 1024], F32, name="cp", tag="cp")
                # r = sign(m - 1) in {-1, +1}: sigma = (r + 1) / 2
                sig = [scp.tile([128, 1024], F32R, name=f"sig{i}", tag=f"sig{i}") for i in range(2)]
                d0 = psd.tile([64, 512], F32, name="d0", tag="d0")
                d1 = psd.tile([64, 512], F32, name="d1", tag="d1")

                neg1 = scp.tile([128, 1], F32, name="neg1", tag="neg1")
                nc.vector.memset(neg1[:], -1.0)
                # cp = c - 0.5 (folds the (r+1)/2 offset into the input)
                nc.vector.tensor_scalar_sub(cp[:], ctile[:], 0.5)

                # t=0 collapses: m_1 = beta*0 + c - spk(-1) = c exactly, so
                # sign and the d-matmuls read ctile directly; m (and the
                # sig ping-pong) first materialize at t=1. Saves two DVE
                # passes and two memsets on the serial scan path.
                nc.scalar.activation(sig[0][:], ctile[:], AF.Sign, bias=neg1[:])
                nc.tensor.matmul(
                    d0[:], wl[:, 0:64], sig[0][:, 0:512], start=True, stop=False
                )
                nc.tensor.matmul(
                    d1[:], wl[:, 0:64], sig[0][:, 512:1024], start=True, stop=False
                )

                for t in range(1, T):
                    rprev = sig[(t + 1) % 2]
                    rcur = sig[t % 2]
                    # u = 0.95*m + (c - 0.5)
                    nc.vector.scalar_tensor_tensor(
                        u[:], (ctile if t == 1 else m)[:], BETA, cp[:],
                        OP.mult, OP.add
                    )
                    # m = -0.5*r_prev + u
                    nc.vector.scalar_tensor_tensor(
                        m[:], rprev[:], -0.5, u[:], OP.mult, OP.add
                    )
                    # r_t = sign(m - 1)  (ACT engine, hidden under DVE)
                    nc.scalar.activation(rcur[:], m[:], AF.Sign, bias=neg1[:])
                    # D += sum_c wl[c,t,hw_w] * r[c,(b,hw_r)]
                    nc.tensor.matmul(
                        d0[:], wl[:, 64 * t : 64 * t + 64], rcur[:, 0:512],
                        start=False, stop=(t == T - 1),
                    )
                    nc.tensor.matmul(
                        d1[:], wl[:, 64 * t : 64 * t + 64], rcur[:, 512:1024],
                        start=False, stop=(t == T - 1),
                    )

                dout = scp.tile([64, 1024], F32, name="dout", tag="dout")
                nc.vector.tensor_copy(dout[:, 0:512], d0[:])
                nc.vector.tensor_copy(dout[:, 512:1024], d1[:])
                nc.sync.dma_start(out_d.ap(), dout[:])

    nc.compile()
    return nc


_NC_CACHE = {}


def _get_nc():
    if "nc" not in _NC_CACHE:
        _NC_CACHE["nc"] = build_nc()
    return _NC_CACHE["nc"]


def host_prep(img, w1, b1, w2, b2, w3, b3, w4, b4,
              g2, bb2, rm2, rv2, g3, bb3, rm3, rv3, g4, bb4, rm4, rv4, wl):
    """Fold BN, build split-fp16 tap tensors + shared input map."""
    s2, sh2 = _fold_bn(_np(g2), _np(bb2), _np(rm2), _np(rv2))
    s3, sh3 = _fold_bn(_np(g3), _np(bb3), _np(rm3), _np(rv3))
    s4, sh4 = _fold_bn(_np(g4), _np(bb4), _np(rm4), _np(rv4))
    for sh, s in ((sh2, s2), (sh3, s3), (sh4, s4)):
        if np.any(sh != 0):
            raise NotImplementedError("nonzero BN shift not supported")
        if np.any(s <= 0):
            raise NotImplementedError("nonpositive BN scale not supported")

    def stack16(taps):
        h, l = _split16(taps)
        return np.ascontiguousarray(np.stack([h, l], axis=0))

    w1t = stack16(_l1_dyrep_taps(_np(w1)))
    w2t = stack16(_block_diag_taps(_np(w2), 4, col_scale=s2))
    w3t = stack16(_block_diag_taps(_np(w3), 2, col_scale=s3))
    w4t = stack16(_block_diag_taps(_np(w4), 1, col_scale=s4))
    biases = [
        _bias_vec(_np(b1), 8),
        _bias_vec(_np(b2) * s2, 4),
        _bias_vec(_np(b3) * s3, 2),
        _bias_vec(_np(b4) * s4, 1),
    ]
    biasp = np.concatenate([b.reshape(1, 128) for b in biases], axis=0)
    biasn = (-0.8 * biasp).astype(np.float32)

    # wl [1, T*128*64] -> [c=128, t, hw=64]
    wlt = np.ascontiguousarray(
        _np(wl).reshape(T, 128, 64).transpose(1, 0, 2).reshape(128, T * 64)
    )
    imgh, imgl = _split16(_np(img))
    return {
        "w1t": w1t, "w2t": w2t, "w3t": w3t, "w4t": w4t,
        "biasp": biasp, "biasn": biasn, "wlt": wlt,
    }, imgh, imgl


def kernel(
    img,
    w1, b1, w2, b2, w3, b3, w4, b4,
    g2, bb2, rm2, rv2, g3, bb3, rm3, rv3, g4, bb4, rm4, rv4,
    wl, bl,
):
    wl = _np(wl)
    bl = _np(bl)
    shared, imgh, imgl = host_prep(
        img, w1, b1, w2, b2, w3, b3, w4, b4,
        g2, bb2, rm2, rv2, g3, bb3, rm3, rv3, g4, bb4, rm4, rv4, wl)

    nc = _get_nc()
    in_maps = [
        {
            **shared,
            "imgh": np.ascontiguousarray(imgh[16 * k : 16 * k + 16]),
            "imgl": np.ascontiguousarray(imgl[16 * k : 16 * k + 16]),
        }
        for k in range(N_CORES)
    ]
    res = run_bass_kernel_spmd(nc, in_maps, list(range(N_CORES)))
    _NC_CACHE["last_res"] = res

    sw = float(np.sum(wl, dtype=np.float64))
    logits = np.empty((B_FULL, 1), np.float32)
    for k in range(N_CORES):
        D = res.results[k]["D"].reshape(64, 16, 64)
        e = np.einsum("hbh->b", D).astype(np.float32)
        logits[16 * k : 16 * k + 16, 0] = (e + sw) * 0.5
    logits += bl.reshape(1, 1)
    return (1.0 / (1.0 + np.exp(-logits))).astype(np.float32)


if __name__ == "__main__":
    nc = build_nc()
    print("built ok")

